# revision 34
# baseline (speedup 1.0000x reference)
"""Trainium2 Bass kernel for nn_ASAPLipsNet (B=4096, IN=128, H=256, OUT=64).

Strategy: pure data parallel over 8 NeuronCores (512 samples each), weights
replicated. Per core:
  forward (fp32, transposed [feat, batch] layout) -> relu masks m1/m2/m3
  per-sample Jacobian chain S3 = W3 @ (m2 * (W2 @ (m1 * W1))) in bf16 on PE,
  ||J_b||^2 via either PE mask-matmuls (q[:, b] += S3sq_seg.T @ m3_col) or
  DVE row-sum reduce + end masking, split to balance engine load,
  out = tanh(softplus(k) * f_out / (||J|| + eps)).
"""
import os
import sys
import types

sys.path.insert(0, "/opt/trn_rl_repo")

import numpy as np
import ml_dtypes


def _install_ntff_hook():
    """antenv in this image lacks axon_hooks; provide it so trace=True works."""
    if "antenv.axon_hooks" in sys.modules:
        return
    mod = types.ModuleType("antenv.axon_hooks")
    state = {"hook": None}

    def set_axon_ntff_profile_hook(h):
        state["hook"] = h

    def get_axon_ntff_profile_hook():
        return state["hook"]

    mod.set_axon_ntff_profile_hook = set_axon_ntff_profile_hook
    mod.get_axon_ntff_profile_hook = get_axon_ntff_profile_hook
    sys.modules["antenv.axon_hooks"] = mod
    try:
        from trn_agent_boot.trn_boot import _ntff_profile_via_ctypes
        set_axon_ntff_profile_hook(_ntff_profile_via_ctypes("/opt/axon/libaxon_pjrt.so"))
    except Exception:
        pass


_install_ntff_hook()

import concourse.bass as bass
import concourse.tile as tile
from concourse import bacc, mybir
from concourse.bass_utils import run_bass_kernel_spmd

F32 = mybir.dt.float32
BF16 = mybir.dt.bfloat16
F8E5 = mybir.dt.float8e5

N_CORES = 8
B = 4096
BC = B // N_CORES          # 512 samples per core
IN, H, OUT = 128, 256, 64
EPS = 1e-4
GS = 4                     # samples per jacobian group (psum free dim 512/128)
NG = BC // GS              # 128 groups per core
GB = 2                     # groups per rhs1 build batch
# groups with g % DVE_RED_MOD == DVE_RED_MOD-1 reduce on DVE, others on PE
DVE_RED_MOD = int(os.environ.get("KERNEL_DVE_RED_MOD", "2"))

_CACHE = {}


def _bcast_cols(ap2d, n_inner):
    """[128, k] AP -> [128, k, n_inner] AP broadcasting along a new inner dim."""
    a = ap2d.rearrange("p (s one) -> p s one", one=1)
    return bass.AP(tensor=a.tensor, offset=a.offset,
                   ap=[list(a.ap[0]), list(a.ap[1]), [0, n_inner]])


def _build_program():
    nc = bacc.Bacc()

    # ---- dram parameters (per-core views; weights replicated) ----
    xT = nc.declare_dram_parameter("xT", [IN, BC], F32, isOutput=False)
    W1T = nc.declare_dram_parameter("W1T", [IN, H], F32, isOutput=False)
    W2T = nc.declare_dram_parameter("W2T", [H, H], F32, isOutput=False)
    W3T = nc.declare_dram_parameter("W3T", [H, H], F32, isOutput=False)
    W1r8 = nc.declare_dram_parameter("W1r8", [H, GB * GS * IN], BF16, isOutput=False)
    W2Tb = nc.declare_dram_parameter("W2Tb", [H, H], BF16, isOutput=False)
    W3Tb = nc.declare_dram_parameter("W3Tb", [H, H], BF16, isOutput=False)
    WaT = nc.declare_dram_parameter("WaT", [H, OUT], F32, isOutput=False)
    b1d = nc.declare_dram_parameter("b1d", [H, 1], F32, isOutput=False)
    b2d = nc.declare_dram_parameter("b2d", [H, 1], F32, isOutput=False)
    b3d = nc.declare_dram_parameter("b3d", [H, 1], F32, isOutput=False)
    baR = nc.declare_dram_parameter("baR", [1, OUT], F32, isOutput=False)
    onesc = nc.declare_dram_parameter("onesc", [128, 1], F32, isOutput=False)
    onesr = nc.declare_dram_parameter("onesr", [1, 128], F32, isOutput=False)
    spv = nc.declare_dram_parameter("spv", [128, 1], F32, isOutput=False)
    out_d = nc.declare_dram_parameter("out", [BC, OUT], F32, isOutput=True)

    with tile.TileContext(nc) as tc:
        with (
            tc.tile_pool(name="consts", bufs=1) as consts,
            tc.tile_pool(name="persist", bufs=1) as persist,
            tc.tile_pool(name="loop", bufs=4) as loop,
            tc.tile_pool(name="fin", bufs=4) as fin,
            tc.tile_pool(name="psA", bufs=3, space="PSUM") as psA,
            tc.tile_pool(name="psB", bufs=4, space="PSUM") as psB,
            tc.tile_pool(name="psQ", bufs=1, space="PSUM") as psQ,
        ):
            # ---- load constants ----
            xT_s = consts.tile([IN, BC], F32, tag="xT")
            nc.sync.dma_start(xT_s[:], xT[:])
            W1T_s = consts.tile([IN, H], F32, tag="W1T")
            nc.sync.dma_start(W1T_s[:], W1T[:])

            def load_chunked(name, dram, rows, cols, dt):
                ts = []
                for c in range(rows // 128):
                    t = consts.tile([128, cols], dt, tag=f"{name}{c}", name=f"{name}{c}")
                    nc.sync.dma_start(t[:], dram[c * 128:(c + 1) * 128, :])
                    ts.append(t)
                return ts

            W2T_s = load_chunked("W2T", W2T, H, H, F32)
            W3T_s = load_chunked("W3T", W3T, H, H, F32)
            W1r8_s = load_chunked("W1r8", W1r8, H, GB * GS * IN, BF16)
            W2Tb_s = load_chunked("W2Tb", W2Tb, H, H, BF16)
            W3Tb_s = load_chunked("W3Tb", W3Tb, H, H, BF16)
            WaT_s = load_chunked("WaT", WaT, H, OUT, F32)
            b1_s = load_chunked("b1", b1d, H, 1, F32)
            b2_s = load_chunked("b2", b2d, H, 1, F32)
            b3_s = load_chunked("b3", b3d, H, 1, F32)
            baR_s = consts.tile([1, OUT], F32, tag="baR")
            nc.sync.dma_start(baR_s[:], baR[:])
            onesc_s = consts.tile([128, 1], F32, tag="onesc")
            nc.sync.dma_start(onesc_s[:], onesc[:])
            onesr_s = consts.tile([1, 128], F32, tag="onesr")
            nc.sync.dma_start(onesr_s[:], onesr[:])
            spv_s = consts.tile([128, 1], F32, tag="spv")
            nc.sync.dma_start(spv_s[:], spv[:])

            # ---- forward pass (fp32, transposed layout [feat, batch]) ----
            # masks stored as single wide tiles [128, 2*BC]: halves indexed by mc
            def fwd_layer(lhsT_tiles, rhs_tiles, bias_tiles, k_chunks, name):
                a_t = []
                mw = persist.tile([128, 2 * BC], BF16, tag=f"m{name}",
                                  name=f"m{name}")
                for mc in range(2):
                    hp = psA.tile([128, BC], F32, tag="mm", name=f"h{name}{mc}")
                    for kc in range(k_chunks):
                        nc.tensor.matmul(
                            hp[:],
                            lhsT_tiles[kc][:, mc * 128:(mc + 1) * 128],
                            rhs_tiles[kc][:],
                            start=(kc == 0),
                            stop=(kc == k_chunks - 1),
                        )
                    a = persist.tile([128, BC], F32, tag=f"a{name}{mc}",
                                     name=f"a{name}{mc}")
                    nc.scalar.activation(a[:], hp[:],
                                         mybir.ActivationFunctionType.Relu,
                                         bias=bias_tiles[mc][:])
                    nc.vector.tensor_scalar(mw[:, mc * BC:(mc + 1) * BC], a[:],
                                            0.0, None, op0=mybir.AluOpType.is_gt)
                    a_t.append(a)
                return a_t, mw

            a1, m1w = fwd_layer([W1T_s], [xT_s], b1_s, 1, "1")

            def build_rhs1(g):
                wide = slice(g * GS, (g + GB) * GS)
                rhs1 = []
                for gc in range(2):
                    r = loop.tile([128, GB * GS * IN], BF16, tag=f"rhs1_{gc}",
                                  name=f"rhs1_{gc}_{g}", bufs=2)
                    nc.gpsimd.tensor_mul(
                        r[:].rearrange("p (s i) -> p s i", s=GB * GS),
                        W1r8_s[gc][:].rearrange("p (s i) -> p s i", s=GB * GS),
                        _bcast_cols(m1w[:, gc * BC + wide.start:
                                        gc * BC + wide.stop], IN))
                    rhs1.append(r)
                return rhs1

            # pre-build the first rhs1 batches on gpsimd while fwd layers 2/3 run
            prebuilt = {g: build_rhs1(g) for g in (0, GB)}

            a2, m2w = fwd_layer(W2T_s, a1, b2_s, 2, "2")
            a3, m3w = fwd_layer(W3T_s, a2, b3_s, 2, "3")

            def mcol(mw, mc, b):
                return mw[:, mc * BC + b: mc * BC + b + 1]

            def mcols(mw, mc, sl):
                return mw[:, mc * BC + sl.start: mc * BC + sl.stop]

            # PE-path accumulator: q[i, b] = sum_h m3[h,b] * S3[b][h, i]^2
            qbank = psQ.tile([128, BC], F32, tag="q", name="qbank")
            nc.vector.memset(qbank[:], 0.0)
            # DVE-path accumulator: Rall[:, mc*BC + b] = rowsums of S3^2
            Rall = persist.tile([128, 2 * BC], F32, tag="Rall", name="Rall")
            nc.gpsimd.memset(Rall[:], 0.0)

            # ---- per-sample Jacobian ----
            rhs1 = None
            for g in range(NG):
                cols = slice(g * GS, (g + 1) * GS)
                # rhs1[gc][h, (gg,s,i)] built per GB-group batch on gpsimd
                if g % GB == 0:
                    rhs1 = prebuilt.pop(g) if g in prebuilt else build_rhs1(g)
                roff = (g % GB) * GS * IN
                # S2 = W2 @ rhs1 (accumulate over gc)
                s2p = []
                for mc in range(2):
                    p = psA.tile([128, GS * IN], F32, tag="mm", name=f"s2_{mc}_{g}")
                    for gc in range(2):
                        nc.tensor.matmul(
                            p[:], W2Tb_s[gc][:, mc * 128:(mc + 1) * 128],
                            rhs1[gc][:, roff:roff + GS * IN],
                            start=(gc == 0), stop=(gc == 1))
                    s2p.append(p)
                # rhs2 = m2 (*) S2  (psum -> sbuf bf16, mask broadcast per segment)
                rhs2 = []
                for mc in range(2):
                    r = loop.tile([128, GS * IN], BF16, tag=f"rhs2_{mc}",
                                  name=f"rhs2_{mc}_{g}")
                    nc.vector.tensor_mul(
                        r[:].rearrange("p (s i) -> p s i", s=GS),
                        s2p[mc][:].rearrange("p (s i) -> p s i", s=GS),
                        _bcast_cols(mcols(m2w, mc, cols), IN))
                    rhs2.append(r)
                # S3 = W3 @ rhs2 (two psum tiles)
                s3p = []
                for mc in range(2):
                    p = psB.tile([128, GS * IN], F32, tag="mm3", name=f"s3_{mc}_{g}")
                    for hc in range(2):
                        nc.tensor.matmul(
                            p[:], W3Tb_s[hc][:, mc * 128:(mc + 1) * 128],
                            rhs2[hc][:], start=(hc == 0), stop=(hc == 1))
                    s3p.append(p)
                # square: psum -> halves of one wide sbuf tile (bf16)
                sqw = loop.tile([128, 2 * GS * IN], BF16, tag="sq", name=f"sq_{g}", bufs=6)
                for mc in range(2):
                    nc.scalar.activation(sqw[:, mc * GS * IN:(mc + 1) * GS * IN],
                                         s3p[mc][:],
                                         mybir.ActivationFunctionType.Square)
                if (g % DVE_RED_MOD == DVE_RED_MOD - 1) if DVE_RED_MOD > 0 \
                        else (g % (-DVE_RED_MOD) != 0):
                    # DVE path: rowsums of both halves in one strided reduce
                    outap = Rall[:].rearrange("p (k b) -> p k b", k=2)[
                        :, :, g * GS:(g + 1) * GS]
                    nc.vector.reduce_sum(
                        outap,
                        sqw[:].rearrange("p (k s i) -> p k s i", k=2, s=GS),
                        axis=mybir.AxisListType.X)
                else:
                    # PE path: q[:, b] += S3sq_seg.T @ m3_col
                    for s in range(GS):
                        b = g * GS + s
                        for mc in range(2):
                            nc.tensor.matmul(
                                qbank[:, b:b + 1],
                                sqw[:, (mc * GS + s) * IN:(mc * GS + s + 1) * IN],
                                mcol(m3w, mc, b),
                                start=(mc == 0), stop=(mc == 1))

            # ---- finalize ----
            qsb = persist.tile([128, BC], F32, tag="qsb", name="qsb")
            nc.vector.tensor_copy(qsb[:], qbank[:])
            Zr = persist.tile([128, 2 * BC], F32, tag="Zr", name="Zr")
            nc.vector.tensor_mul(Zr[:], Rall[:], m3w[:])

            qp_full = psB.tile([128, GS * IN], F32, tag="mm3", name="qp_full")
            qp = qp_full[:, 0:4]
            for bt in range(4):
                bsl = slice(bt * 128, (bt + 1) * 128)
                nc.tensor.matmul(qp[:, bt:bt + 1], qsb[:, bsl], onesc_s[:],
                                 start=True, stop=False)
                for mc in range(2):
                    nc.tensor.matmul(
                        qp[:, bt:bt + 1],
                        Zr[:, mc * BC + bt * 128: mc * BC + (bt + 1) * 128],
                        onesc_s[:], start=False, stop=(mc == 1))

            fp_full = psA.tile([128, BC], F32, tag="mm", name="fp_full")
            fp = fp_full[:, 0:4 * OUT]
            for bt in range(4):
                bsl = slice(bt * 128, (bt + 1) * 128)
                fsl = fp[:, bt * OUT:(bt + 1) * OUT]
                for hc in range(2):
                    nc.tensor.matmul(fsl, a3[hc][:, bsl], WaT_s[hc][:],
                                     start=(hc == 0), stop=False)
                nc.tensor.matmul(fsl, onesr_s[:], baR_s[:],
                                 start=False, stop=True)

            for bt in range(4):
                sq = fin.tile([128, 1], F32, tag="nrm", name=f"nrm{bt}")
                nc.scalar.activation(sq[:], qp[:, bt:bt + 1],
                                     mybir.ActivationFunctionType.Sqrt)
                se = fin.tile([128, 1], F32, tag="se", name=f"se{bt}")
                nc.vector.tensor_scalar_add(se[:], sq[:], EPS)
                rc = fin.tile([128, 1], F32, tag="rc", name=f"rc{bt}")
                nc.vector.reciprocal(rc[:], se[:])
                sc = fin.tile([128, 1], F32, tag="sc", name=f"sc{bt}")
                nc.vector.tensor_mul(sc[:], rc[:], spv_s[:])
                ot = fin.tile([128, OUT], F32, tag="ot", name=f"ot{bt}")
                nc.scalar.activation(ot[:], fp[:, bt * OUT:(bt + 1) * OUT],
                                     mybir.ActivationFunctionType.Tanh,
                                     scale=sc[:])
                nc.sync.dma_start(out_d[bt * 128:(bt + 1) * 128, :], ot[:])

    nc.finalize()
    return nc


def _get_program():
    if "nc" not in _CACHE:
        _CACHE["nc"] = _build_program()
    return _CACHE["nc"]


def kernel(x, W1, b1, W2, b2, W3, b3, Wa, ba, k):
    x = np.asarray(x, dtype=np.float32)
    W1 = np.asarray(W1, dtype=np.float32)
    W2 = np.asarray(W2, dtype=np.float32)
    W3 = np.asarray(W3, dtype=np.float32)
    Wa = np.asarray(Wa, dtype=np.float32)
    b1 = np.asarray(b1, dtype=np.float32)
    b2 = np.asarray(b2, dtype=np.float32)
    b3 = np.asarray(b3, dtype=np.float32)
    ba = np.asarray(ba, dtype=np.float32)
    k = np.asarray(k, dtype=np.float32)

    sp = np.logaddexp(0.0, k[0]).astype(np.float32)  # stable softplus
    W1b = W1.astype(ml_dtypes.bfloat16)

    shared = {
        "W1T": np.ascontiguousarray(W1.T),
        "W2T": np.ascontiguousarray(W2.T),
        "W3T": np.ascontiguousarray(W3.T),
        "W1r8": np.ascontiguousarray(np.tile(W1b, (1, GB * GS))),
        "W2Tb": np.ascontiguousarray(W2.T).astype(ml_dtypes.bfloat16),
        "W3Tb": np.ascontiguousarray(W3.T).astype(ml_dtypes.bfloat16),
        "WaT": np.ascontiguousarray(Wa.T),
        "b1d": b1.reshape(H, 1),
        "b2d": b2.reshape(H, 1),
        "b3d": b3.reshape(H, 1),
        "baR": ba.reshape(1, OUT),
        "onesc": np.ones((128, 1), np.float32),
        "onesr": np.ones((1, 128), np.float32),
        "spv": np.full((128, 1), sp, np.float32),
    }
    in_maps = []
    for c in range(N_CORES):
        shard = x[c * BC:(c + 1) * BC]
        m = dict(shared)
        m["xT"] = np.ascontiguousarray(shard.T)
        in_maps.append(m)

    nc = _get_program()
    trace = bool(int(os.environ.get("KERNEL_TRACE", "0")))
    r = run_bass_kernel_spmd(nc, in_maps, list(range(N_CORES)), trace=trace)
    if trace:
        kernel.last_exec_time_ns = r.exec_time_ns
        kernel.last_results = r
    out = np.concatenate([r.results[c]["out"] for c in range(N_CORES)], axis=0)
    return out.astype(np.float32)


kernel.last_exec_time_ns = None
